# revision 2
# baseline (speedup 1.0000x reference)
"""Trainium2 Bass kernel for nn_ChannelAttentionLayer — v2.

Math (vs baseline kernel.py):
  - Conv biases cancel under batch-stats BN -> dropped.
  - Pad-ring keys: K_raw = V_raw = 0 there; K's BN offset shifts every
    query's scores equally (softmax-over-keys invariant), so ring keys get
    attention weight ~e^-70 -> dropped entirely.  Attention runs over the
    4096 interior keys = exactly 32 tiles of 128.
  - K needs NO elementwise normalization: softmax_k(Kbn^T Qbn) =
    softmax_k(Kraw^T Q^) with Q^ = (aq*ak) o Qraw + (cq*ak).  Only Q gets
    the affine; K raw conv output feeds the scores matmul directly.
  - V BN affine folded into the output epilogue (attn columns sum to 1).
  - Softmax denominator via an all-ones column appended to V^T.

Sharding: 8 cores = 4 batches x 2 query-halves.  Inputs per core:
  xq [128,CT,2244]  padded rows h*32..h*32+33 (Q conv input)
  xi [128,CT,2048]  the OTHER half's 32 interior rows (keys 0..2047); this
                    core's own 32 query rows (keys 2048..4095) are carved
                    out of xq on-chip with strided SBUF->SBUF DMAs on the
                    second HWDGE queue, deduplicating 2.1MB of HBM traffic
  wq [128,9,CT,256], wk/wv [128,CT,256], vecs [128,12]
BatchNorm statistics are combined with a tiny AllReduce (Q/K) and a second
one for V; V^T squares run on the idle Pool engine, which also takes half
of each output epilogue.
"""
import math

import numpy as np

import concourse.bass as bass
import concourse.mybir as mybir
import concourse.tile as tile
from concourse.bass_utils import run_bass_kernel_spmd

dt = mybir.dt
AF = mybir.ActivationFunctionType
ALU = mybir.AluOpType
F32 = dt.float32
F32R = dt.float32r

N_CORES = 8
CT = 2                   # channel tiles (256 = 2 x 128)
H = W = 64
HP = 66
NPOS = H * W             # 4096 interior positions (= keys per batch)
NPAD = HP * HP           # 4356 padded positions (reference BN count for K/V)
NKT = 32                 # key tiles: 4096 = 32 x 128
QSH = 2048               # query positions per core
QROWS = 34 * HP          # 2244: xq length
CSHIFT = 104.0           # softmax shift; global max score is ~101
EPS = 1e-5
NQ_TOT = float(4 * NPOS)
NKV_TOT2 = float(2 * 4 * NPAD)   # x2: both half-cores contribute full sums

# ---------------------------------------------------------------------------
# Workaround: this walrus build rejects >1 semaphore wait per instruction.
_UID = [0]


def _split_waits_in_module(nc):
    for fn in nc.m.functions:
        for blk in fn.blocks:
            insts = list(blk.instructions)
            if not any(
                i.sync_info and i.sync_info.on_wait and len(i.sync_info.on_wait) > 1
                for i in insts
            ):
                continue
            new = []
            for inst in insts:
                si = inst.sync_info
                waits = list(si.on_wait) if (si and si.on_wait) else []
                if len(waits) > 1:
                    for w in waits[:-1]:
                        _UID[0] += 1
                        new.append(
                            mybir.InstNoOp(
                                name=f"I-waitsplit-{_UID[0]}",
                                engine=inst.engine,
                                ins=[],
                                outs=[],
                                sync_info=mybir.SyncInfo(on_wait=[w], on_update=[]),
                            )
                        )
                    inst.sync_info = mybir.SyncInfo(
                        on_wait=waits[-1:], on_update=list(si.on_update or [])
                    )
                new.append(inst)
            del blk.instructions[:]
            for i in new:
                blk.instructions.append(i)


class TC(tile.TileContext):
    def __exit__(self, exc_type, exc_val, exc_tb):
        r = super().__exit__(exc_type, exc_val, exc_tb)
        if exc_type is None:
            _split_waits_in_module(self.nc)
        return r


# ---------------------------------------------------------------------------
def build_nc(reps: int = 1, skip_cc: bool = False):
    nc = bass.Bass("TRN2", target_bir_lowering=False, num_devices=N_CORES)

    xq_d = nc.dram_tensor("xq", [128, CT, QROWS], F32R, kind="ExternalInput")
    xi_d = nc.dram_tensor("xi", [128, CT, NPOS], F32R, kind="ExternalInput")
    wq_d = nc.dram_tensor("wq", [128, 9, CT, 256], F32R, kind="ExternalInput")
    wk_d = nc.dram_tensor("wk", [128, CT, 256], F32R, kind="ExternalInput")
    wv_d = nc.dram_tensor("wv", [128, CT, 256], F32R, kind="ExternalInput")
    vec_d = nc.dram_tensor("vecs", [128, 12], F32, kind="ExternalInput")
    y_d = nc.dram_tensor("y", [16, 128, 256], F32, kind="ExternalOutput")

    cc_in1 = nc.dram_tensor("cc_in1", [128, 8], F32)
    cc_out1 = nc.dram_tensor("cc_out1", [128, 8], F32, addr_space="Shared")
    cc_in2 = nc.dram_tensor("cc_in2", [128, 4], F32)
    cc_out2 = nc.dram_tensor("cc_out2", [128, 4], F32, addr_space="Shared")
    scr_ab = nc.dram_tensor("scr_ab", [512], F32)
    scr_v = nc.dram_tensor("scr_v", [512], F32)

    # Q conv row blocks: grid rows 1..32 grouped (7,7,7,7,4).
    QBLK = [(1, 7), (8, 7), (15, 7), (22, 7), (29, 4)]
    NKBLK = 4            # K conv: 4 blocks of 1024 interior positions per mt

    with TC(nc) as tc:
        with (
            tc.tile_pool(name="sb_in", bufs=1) as sb_in,
            tc.tile_pool(name="sb_w", bufs=1) as sb_w,
            tc.tile_pool(name="sb_small", bufs=1) as sb_small,
            tc.tile_pool(name="sb_tmp", bufs=3) as sb_tmp,
            tc.tile_pool(name="sb_e", bufs=5) as sb_e,
            tc.tile_pool(name="sb_out", bufs=4) as sb_out,
            tc.tile_pool(name="ps_s2", bufs=2, space="PSUM") as ps_s2,
            tc.tile_pool(name="psb1", bufs=4, space="PSUM") as psb1,
        ):
            def body(_it):
                f = F32

                # ------- loads: one bulk stream on the SP HWDGE queue,
                # ordered to match PE consumption: other-half keys (V segA +
                # K b0/b1) -> wq h0 + xq (Q conv) -> own-half keys (K b2/b3
                # + V segB, not needed until after Q mt0) -> wq h1.  Small
                # weights ride the Pool SWDGE queue concurrently.
                # (SWDGE InstTriggerDma doesn't lower inside For_i loops, so
                # the timing build falls back to the SP queue for these)
                wdma = nc.gpsimd if reps == 1 else nc.sync
                wv = sb_in.tile([128, CT, 256], F32R, tag="wv")
                wdma.dma_start(out=wv, in_=wv_d.ap())
                wk = sb_in.tile([128, CT, 256], F32R, tag="wk")
                wdma.dma_start(out=wk, in_=wk_d.ap())
                vecs = sb_in.tile([128, 12], f, tag="vecs")
                wdma.dma_start(out=vecs, in_=vec_d.ap())
                xi = sb_in.tile([128, CT, NPOS], F32R, tag="xi")
                xq = sb_in.tile([128, CT, QROWS], F32R, tag="xq")
                wq = sb_in.tile([128, 9, CT, 256], F32R, tag="wq")
                HQ = 17 * HP
                for lo, hi in ((0, 256), (256, 512), (512, 1024),
                               (1024, 2048)):
                    for ci in range(CT):
                        nc.sync.dma_start(
                            out=xi[:, ci, lo:hi], in_=xi_d.ap()[:, ci, lo:hi]
                        )
                nc.sync.dma_start(out=wq[:, :, :, 0:128], in_=wq_d.ap()[:, :, :, 0:128])
                for ci in range(CT):
                    nc.sync.dma_start(
                        out=xq[:, ci, 0:HQ], in_=xq_d.ap()[:, ci, 0:HQ]
                    )
                for ci in range(CT):
                    nc.sync.dma_start(
                        out=xi[:, ci, 2048:3072], in_=xi_d.ap()[:, ci, 2048:3072]
                    )
                for ci in range(CT):
                    nc.sync.dma_start(
                        out=xq[:, ci, HQ:QROWS], in_=xq_d.ap()[:, ci, HQ:QROWS]
                    )
                for ci in range(CT):
                    nc.sync.dma_start(
                        out=xi[:, ci, 3072:4096], in_=xi_d.ap()[:, ci, 3072:4096]
                    )
                nc.sync.dma_start(out=wq[:, :, :, 128:256], in_=wq_d.ap()[:, :, :, 128:256])

                consts = sb_small.tile([128, 2], f, tag="consts")
                nc.vector.memset(consts[:, 0:1], EPS)
                nc.vector.memset(consts[:, 1:2], -CSHIFT)
                eps_t = consts[:, 0:1]
                negc_t = consts[:, 1:2]
                invn8 = sb_small.tile([128, 8], f, tag="invn8")
                nc.vector.memset(invn8[:, 0:2], 1.0 / NQ_TOT)
                nc.vector.memset(invn8[:, 2:4], 1.0 / NKV_TOT2)
                nc.vector.memset(invn8[:, 4:6], 1.0 / NQ_TOT)
                nc.vector.memset(invn8[:, 6:8], 1.0 / NKV_TOT2)

                qraw = sb_w.tile([128, CT, QSH], F32R, tag="qraw")
                kraw = sb_w.tile([128, CT, NPOS], F32R, tag="kraw")
                vt = sb_w.tile([128, NKT, 258], F32R, tag="vt")
                sums = sb_small.tile([128, 8], f, tag="sums")
                sums_v = sb_small.tile([128, 4], f, tag="sums_v")
                # per-block stat partials: Q sum 0:5, Q sumsq 5:10,
                # K sum 10:14, K sumsq 14:18
                qk_part = sb_small.tile([128, CT, 18], f, tag="qk_part")

                # ------- V^T conv (keys on partitions) + V stats -----------
                # vt[kt] rows = interior positions kt*128..kt*128+127.
                # V statistics: squares on the idle Pool engine, running
                # per-partition accumulators on DVE, and a single final
                # cross-partition ones-matmul pair (512 PE rows total).
                ones_r = sb_small.tile([128, 1], F32R, tag="ones_r")
                nc.vector.tensor_copy(
                    out=ones_r, in_=nc.const_aps.tensor(1.0, (128, 1), F32)
                )
                vsum_acc = sb_small.tile([128, 256], F32R, tag="vsum_acc")
                vsq_acc = sb_small.tile([128, 256], F32R, tag="vsq_acc")
                nc.vector.tensor_copy(
                    out=vsum_acc, in_=nc.const_aps.tensor(0.0, (128, 256), F32)
                )
                nc.vector.tensor_copy(
                    out=vsq_acc, in_=nc.const_aps.tensor(0.0, (128, 256), F32)
                )

                vstat_deferred = []

                def vt_tile(kt, defer=False):
                    pvt = psb1.tile([128, 258], f, tag="b1", name=f"pvt{kt}")
                    for ci in range(CT):
                        nc.tensor.matmul(
                            pvt[:, 0:256],
                            xi[:, ci, kt * 128:(kt + 1) * 128],
                            wv[:, ci, :],
                            start=(ci == 0), stop=(ci == CT - 1),
                        )
                    nc.scalar.activation(
                        out=vt[:, kt, 0:256], in_=pvt[:, 0:256], func=AF.Copy,
                    )

                    def stats(kt=kt, defer=defer):
                        vt2 = sb_tmp.tile([128, 256], F32R, tag="vsq",
                                          name=f"vt2_{kt}", bufs=3)
                        sq_eng = nc.gpsimd if (defer or kt % 2 == 0) \
                            else nc.vector
                        sq_eng.tensor_mul(
                            vt2, vt[:, kt, 0:256], vt[:, kt, 0:256]
                        )
                        nc.vector.tensor_add(vsum_acc, vsum_acc,
                                             vt[:, kt, 0:256])
                        nc.vector.tensor_add(vsq_acc, vsq_acc, vt2)

                    # segB stat ops are deferred past the (latency-critical)
                    # BN affine + Q normalize so the in-order DVE queue
                    # doesn't block them behind 6+us of accumulator chain.
                    if defer:
                        vstat_deferred.append(stats)
                    else:
                        stats()

                # ------- K conv (1x1 over interior) -> kraw stays RAW ------
                def emit_k_block(mt, bi):
                    sft = bi * 1024
                    pk = ps_s2.tile([128, 1024], f, tag="s2", name=f"pk{mt}{bi}")
                    for ci in range(CT):
                        for sub in range(0, 1024, 512):
                            nc.tensor.matmul(
                                pk[:, sub:sub + 512],
                                wk[:, ci, mt * 128:(mt + 1) * 128],
                                xi[:, ci, sft + sub:sft + sub + 512],
                                start=(ci == 0), stop=(ci == CT - 1),
                            )
                    nc.scalar.activation(
                        out=kraw[:, mt, sft:sft + 1024], in_=pk[:, 0:1024],
                        func=AF.Copy,
                        accum_out=qk_part[:, mt, 10 + bi:11 + bi],
                    )
                    # K sumsq: blocks 0/1 as an ACT Square pass (accum_out);
                    # blocks 2/3 (which share their phase with Q conv's ACT
                    # evictions) square on the idle Pool engine + DVE reduce,
                    # keeping both ACT and DVE shallow for the stats chain.
                    scr = sb_tmp.tile([128, 1024], f, tag="tmp",
                                      name=f"ksq{mt}_{bi}")
                    if bi < 2:
                        nc.scalar.activation(
                            out=scr, in_=kraw[:, mt, sft:sft + 1024],
                            func=AF.Square,
                            accum_out=qk_part[:, mt, 14 + bi:15 + bi],
                        )
                    else:
                        nc.gpsimd.tensor_mul(
                            scr, kraw[:, mt, sft:sft + 1024],
                            kraw[:, mt, sft:sft + 1024]
                        )
                        nc.vector.reduce_sum(
                            out=qk_part[:, mt, 14 + bi:15 + bi], in_=scr,
                            axis=mybir.AxisListType.X,
                        )

                # ------- Q conv (3x3, 9 shifted spans; wrap garbage lands in
                # cols 64/65 of each 66-wide row, dropped at eviction).
                # Sum accumulates at eviction; Square pass right after.
                def emit_q_block(mt, r0, nr):
                    n = nr * HP - 2
                    pq = ps_s2.tile([128, 512], f, tag="s2", name=f"pq{mt}{r0}")
                    first = True
                    for tap in range(9):
                        ty, tx = tap // 3, tap % 3
                        sft = (r0 + ty - 1) * HP + tx
                        nc.tensor.matmul(
                            pq[:, 0:n],
                            wq[:, tap, 0, mt * 128:(mt + 1) * 128],
                            xq[:, 0, sft:sft + n],
                            start=first, stop=False,
                        )
                        nc.tensor.matmul(
                            pq[:, 0:n],
                            wq[:, tap, 1, mt * 128:(mt + 1) * 128],
                            xq[:, 1, sft:sft + n],
                            start=False, stop=(tap == 8),
                        )
                        first = False
                    bi = QBLK.index((r0, nr))
                    qsl = qraw[:, mt, (r0 - 1) * 64:(r0 - 1 + nr) * 64]
                    nc.scalar.activation(
                        out=qsl.rearrange("p (a b) -> p a b", a=nr),
                        in_=pq[:, 0:nr * HP]
                        .rearrange("p (a b) -> p a b", a=nr)[:, :, 0:64],
                        func=AF.Copy,
                        accum_out=qk_part[:, mt, bi:bi + 1],
                    )
                    scr = sb_tmp.tile([128, 512], f, tag="tmp",
                                      name=f"qsq{mt}_{r0}")
                    nc.scalar.activation(
                        out=scr[:, 0:nr * 64], in_=qsl, func=AF.Square,
                        accum_out=qk_part[:, mt, 5 + bi:6 + bi],
                    )

                # gathers: DVE free-axis reduces; emitted as soon as the
                # feeding partials are complete so only the last Q-mt1 pair
                # sits on the AllReduce critical path.
                def gather(ci, lo, hi, col):
                    nc.vector.reduce_sum(
                        out=sums[:, col:col + 1], in_=qk_part[:, ci, lo:hi],
                        axis=mybir.AxisListType.X,
                    )

                # ------- emission: V^T segment A interleaved with K blocks
                # over the shipped keys (0:2048) while xi streams in; then
                # Q mt0, K b2/b3 (copied keys), Q mt1.
                for g in range(4):
                    for kt in range(4 * g, 4 * g + 4):
                        vt_tile(kt)
                    if g >= 2:
                        for mt in range(2):
                            emit_k_block(mt, g - 2)

                for r0, nr in QBLK:
                    emit_q_block(0, r0, nr)
                gather(0, 0, 5, 0)
                gather(0, 5, 10, 4)
                for mt in range(2):
                    for bi in (2, 3):
                        emit_k_block(mt, bi)
                for ci in range(CT):
                    gather(ci, 10, 14, 2 + ci)
                    gather(ci, 14, 18, 6 + ci)
                for r0, nr in QBLK:
                    emit_q_block(1, r0, nr)
                gather(1, 0, 5, 1)
                gather(1, 5, 10, 5)

                # ---- AllReduce #1: Q/K stats (critical path) ----
                nc.sync.dma_start(out=cc_in1[:, :], in_=sums)
                sums_g = sb_small.tile([128, 8], f, tag="sums_g")
                if skip_cc:
                    nc.sync.dma_start(out=sums_g, in_=cc_in1[:, :])
                else:
                    nc.gpsimd.collective_compute(
                        "AllReduce", ALU.add,
                        replica_groups=[list(range(N_CORES))],
                        ins=[cc_in1.ap().opt()], outs=[cc_out1.ap().opt()],
                    )
                    nc.sync.dma_start(out=sums_g, in_=cc_out1[:, :])

                # ---- V^T second segment: PE filler during the roundtrip ----
                for kt in range(16, NKT):
                    vt_tile(kt, defer=True)
                nc.vector.tensor_copy(
                    out=vt[:, :, 256:257],
                    in_=nc.const_aps.tensor(1.0, (128, NKT, 1), F32),
                )
                nc.vector.tensor_copy(
                    out=vt[:, :, 257:258],
                    in_=nc.const_aps.tensor(0.0, (128, NKT, 1), F32),
                )

                # ---- V stats finale + AllReduce #2 + V affine/broadcast.
                # Emitted a few pairs INTO the attention stream so the PE
                # ones-matmuls never wait on the Pool/DVE accumulator chain,
                # which trails the segB convs by several us.  Only the
                # epilogue (~25us later) needs the result.
                avcv = sb_small.tile([128, 4], f, tag="avcv")
                avcv_b = sb_small.tile([128, 512], f, tag="avcv_b")
                av_b = avcv_b[:, 0:256]
                cv_b = avcv_b[:, 256:512]
                sums_vg = sb_small.tile([128, 4], f, tag="sums_vg")

                def emit_v_finale():
                    pvs = ps_s2.tile([1, 512], f, tag="s2", name="pvs")
                    nc.tensor.matmul(pvs[0:1, 0:256], ones_r, vsum_acc,
                                     start=True, stop=True)
                    nc.tensor.matmul(pvs[0:1, 256:512], ones_r, vsq_acc,
                                     start=True, stop=True)
                    vrow = sb_small.tile([1, 512], f, tag="vrow")
                    nc.vector.tensor_copy(out=vrow, in_=pvs)
                    nc.sync.dma_start(out=scr_v.ap(), in_=vrow)
                    nc.sync.dma_start(
                        out=sums_v,
                        in_=bass.AP(tensor=scr_v, offset=0,
                                    ap=[[1, 128], [256, 2], [128, 2]]),
                    )
                    nc.sync.dma_start(out=cc_in2[:, :], in_=sums_v)
                    if skip_cc:
                        nc.sync.dma_start(out=sums_vg, in_=cc_in2[:, :])
                    else:
                        nc.gpsimd.collective_compute(
                            "AllReduce", ALU.add,
                            replica_groups=[list(range(N_CORES))],
                            ins=[cc_in2.ap().opt()], outs=[cc_out2.ap().opt()],
                        )
                        nc.sync.dma_start(out=sums_vg, in_=cc_out2[:, :])
                    # V affine: av = gv*rsqrt(var+eps), cv = betav - av*mean
                    mv = sb_small.tile([128, 2], f, tag="mv")
                    vv = sb_small.tile([128, 2], f, tag="vv")
                    av = avcv[:, 0:2]
                    cv = avcv[:, 2:4]
                    nc.vector.tensor_scalar_mul(mv, sums_vg[:, 0:2],
                                                1.0 / NKV_TOT2)
                    nc.vector.tensor_scalar_mul(vv, sums_vg[:, 2:4],
                                                1.0 / NKV_TOT2)
                    nc.vector.tensor_mul(av, mv, mv)
                    nc.vector.tensor_sub(vv, vv, av)
                    nc.scalar.activation(out=vv, in_=vv, func=AF.Ln, bias=eps_t)
                    nc.scalar.activation(out=av, in_=vv, func=AF.Exp, scale=-0.5)
                    nc.vector.tensor_mul(av, vecs[:, 4:6], av)
                    nc.vector.tensor_mul(cv, av, mv)
                    nc.vector.tensor_sub(cv, vecs[:, 10:12], cv)
                    nc.sync.dma_start(
                        out=bass.AP(tensor=scr_ab, offset=0,
                                    ap=[[1, 128], [128, 4]]),
                        in_=avcv,
                    )
                    nc.sync.dma_start(
                        out=avcv_b,
                        in_=bass.AP(tensor=scr_ab, offset=0,
                                    ap=[[0, 128], [1, 512]]),
                    )

                # -------- Q/K affine, batched over 4 cols (q0,q1,k0,k1):
                # a = gamma * exp(-0.5*ln(var+eps)); c = beta - a*mean;
                # then fold K into Q: a' = aq*ak, c' = cq*ak.
                mm8 = sb_small.tile([128, 8], f, tag="mm8")
                var4 = sb_small.tile([128, 4], f, tag="var4")
                a4 = sb_small.tile([128, 4], f, tag="a4")
                cq2 = sb_small.tile([128, 2], f, tag="cq2")
                ap2 = sb_small.tile([128, 2], f, tag="ap2")
                cp2 = sb_small.tile([128, 2], f, tag="cp2")
                nc.vector.tensor_mul(mm8, sums_g, invn8)
                mean4 = mm8[:, 0:4]
                nc.vector.tensor_mul(var4, mean4, mean4)
                nc.vector.tensor_sub(var4, mm8[:, 4:8], var4)
                nc.scalar.activation(out=var4, in_=var4, func=AF.Ln, bias=eps_t)
                nc.scalar.activation(out=a4, in_=var4, func=AF.Exp, scale=-0.5)
                nc.vector.tensor_mul(a4, vecs[:, 0:4], a4)
                nc.vector.tensor_mul(cq2, a4[:, 0:2], mean4[:, 0:2])
                nc.vector.tensor_sub(cq2, vecs[:, 6:8], cq2)
                nc.vector.tensor_mul(ap2, a4[:, 0:2], a4[:, 2:4])
                nc.vector.tensor_mul(cp2, cq2, a4[:, 2:4])

                # ------- normalize Q in place (f32r): Q^ = a' o Q + c'.
                # Chunk 0 (the only one the first scores matmul needs) runs
                # on ACT right behind the affine's Ln/Exp; the rest and the
                # deferred segB V-stat ops follow on DVE.
                for j in range(4):
                    for ci in range(CT):
                        qsl = qraw[:, ci, j * 512:(j + 1) * 512]
                        if j == 0:
                            nc.scalar.activation(
                                out=qsl, in_=qsl, func=AF.Identity,
                                bias=cp2[:, ci:ci + 1], scale=ap2[:, ci:ci + 1],
                            )
                        else:
                            nc.vector.tensor_scalar(
                                qsl, qsl, ap2[:, ci:ci + 1], cp2[:, ci:ci + 1],
                                ALU.mult, ALU.add,
                            )

                # ---------------- attention ----------------
                # Flat software pipeline over (qb, kt-pair): the E->V matmuls
                # trail one pair behind so the exp latency stays off the PE
                # critical path, including across qb boundaries.
                PAIRS = [(2 * p, 2 * p + 2) for p in range(NKT // 2)]
                # last qb: finish with two single-kt pairs so the final
                # exp->out->epilogue chain into the drain is half as deep
                PAIRS_LAST = PAIRS[:-1] + [(30, 31), (31, 32)]
                po = {}

                def emit_out(qb, k0, k1, e2):
                    for kt in range(k0, k1):
                        off = (kt - k0) * 512
                        for qt in range(4):
                            nc.tensor.matmul(
                                po[qb][qt],
                                e2[:, off + qt * 128:off + (qt + 1) * 128],
                                vt[:, kt, :],
                                start=(kt == 0), stop=(kt == NKT - 1),
                            )

                def emit_epilogue(qb):
                    # recip on DVE; ACT scale-evicts (freeing PSUM fast);
                    # qt3's affine goes to Pool so the DVE chain and the
                    # last y DMA finish ~in parallel; y ships as two
                    # 2-tile DMAs (SP dispatch is 650ns a pop).
                    for pr in range(2):
                        ot2 = sb_out.tile([128, 2, 256], f, tag="ot",
                                          name=f"ot{qb}_{pr}", bufs=2)
                        for sub in range(2):
                            qt = pr * 2 + sub
                            qg = qb * 4 + qt
                            rd = sb_small.tile([128, 1], f, tag="rd",
                                               name=f"r{qg}", bufs=4)
                            nc.vector.reciprocal(out=rd,
                                                 in_=po[qb][qt][:, 256:257])
                            ot = ot2[:, sub, :]
                            nc.scalar.activation(
                                out=ot, in_=po[qb][qt][:, 0:256], func=AF.Copy,
                                scale=rd,
                            )
                            eng = nc.gpsimd if qt == 3 else nc.vector
                            eng.tensor_mul(ot, ot, av_b)
                            eng.tensor_add(ot, ot, cv_b)
                        nc.sync.dma_start(
                            out=bass.AP(
                                tensor=y_d,
                                offset=(qb * 4 + pr * 2) * 128 * 256,
                                ap=[[256, 128], [128 * 256, 2], [1, 256]],
                            ),
                            in_=ot2,
                        )

                pend = None
                for qb in range(4):
                    po[qb] = [
                        psb1.tile([128, 258], f, tag="b1", name=f"po{qb}_{i}")
                        for i in range(4)
                    ]
                    for (k0, k1) in (PAIRS_LAST if qb == 3 else PAIRS):
                        w = (k1 - k0) * 512
                        ps_s = ps_s2.tile([128, 1024], f, tag="s2",
                                          name=f"ps{qb}_{k0}")
                        for kt in range(k0, k1):
                            off = (kt - k0) * 512
                            for ci in range(CT):
                                nc.tensor.matmul(
                                    ps_s[:, off:off + 512],
                                    kraw[:, ci, kt * 128:(kt + 1) * 128],
                                    qraw[:, ci, qb * 512:(qb + 1) * 512],
                                    start=(ci == 0), stop=(ci == CT - 1),
                                )
                        e2 = sb_e.tile([128, 1024], F32R, tag="e",
                                       name=f"e{qb}_{k0}")
                        nc.scalar.activation(
                            out=e2[:, 0:w], in_=ps_s[:, 0:w], func=AF.Exp,
                            bias=negc_t,
                        )
                        if pend is not None:
                            emit_out(*pend)
                            if pend[2] == NKT:      # last pair of its qb
                                emit_epilogue(pend[0])
                        pend = (qb, k0, k1, e2)
                        if qb == 0 and k0 == 8:
                            # segB V-stat ops, deferred past the affine +
                            # normalize execution window
                            for stats in vstat_deferred:
                                stats()
                        if qb == 0 and k0 == 24:
                            emit_v_finale()
                emit_out(*pend)
                emit_epilogue(pend[0])

            if reps == 1:
                body(0)
            else:
                with tc.For_i(0, reps, 1) as it:
                    body(it)
    return nc


# ---------------------------------------------------------------------------
def _prep_inputs(x, Wq, Wk, Wv, gq, betaq, gk, betak, gv, betav):
    """Build the 8 per-core input maps (all fp32, pre-laid-out)."""
    x = np.asarray(x, np.float32)
    B = x.shape[0]
    xp_full = np.zeros((B, 256, HP, HP), np.float32)
    xp_full[:, :, 1:65, 1:65] = x

    wq_t = np.ascontiguousarray(
        np.asarray(Wq, np.float32).reshape(256, CT, 128, 3, 3)
        .transpose(2, 3, 4, 1, 0)
    ).reshape(128, 9, CT, 256)
    wk_t = np.ascontiguousarray(
        np.asarray(Wk, np.float32).reshape(256, CT, 128).transpose(2, 1, 0)
    )
    wv_t = np.ascontiguousarray(
        np.asarray(Wv, np.float32).reshape(256, CT, 128).transpose(2, 1, 0)
    )
    cols = [np.asarray(v, np.float32).reshape(CT, 128).T
            for v in (gq, gk, gv, betaq, betak, betav)]
    vecs = np.concatenate(cols, axis=1).astype(np.float32)  # (128, 12)
    vecs = np.ascontiguousarray(vecs)

    in_maps = []
    for core in range(N_CORES):
        b, h = core // 2, core % 2
        xq_b = np.ascontiguousarray(
            xp_full[b][:, h * 32:h * 32 + 34, :]
            .reshape(CT, 128, QROWS).transpose(1, 0, 2)
        )
        # keys 0:2048 = the OTHER half's 32 interior rows; keys 2048:4096 =
        # this core's own query rows (consumed late -> shipped late)
        oh = (1 - h) * 32
        xr = np.concatenate(
            [x[b][:, oh:oh + 32, :], x[b][:, h * 32:h * 32 + 32, :]], axis=1
        )
        xi_b = np.ascontiguousarray(
            xr.reshape(CT, 128, NPOS).transpose(1, 0, 2)
        )
        in_maps.append({
            "xq": xq_b, "xi": xi_b, "wq": wq_t, "wk": wk_t, "wv": wv_t,
            "vecs": vecs,
        })
    return in_maps


_NC_CACHE = {}


def _get_nc(reps=1, skip_cc=False):
    key = (reps, skip_cc)
    if key not in _NC_CACHE:
        _NC_CACHE[key] = build_nc(reps, skip_cc)
    return _NC_CACHE[key]


def _assemble(results):
    out = np.empty((4, 256, 4096), np.float32)
    for core, r in enumerate(results):
        b, h = core // 2, core % 2
        yc = r["y"].reshape(QSH, 256)          # (q, oc)
        out[b, :, h * QSH:(h + 1) * QSH] = yc.T
    return out.reshape(4, 256, 64, 64)


def kernel(x, Wq, bq, gq, betaq, Wk, bk, gk, betak, Wv, bv, gv, betav,
           _reps=1):
    # bq/bk/bv are mathematically irrelevant: BatchNorm with batch statistics
    # removes any per-channel constant shift (including the pad-ring bias).
    in_maps = _prep_inputs(x, Wq, Wk, Wv, gq, betaq, gk, betak, gv, betav)
    nc = _get_nc(_reps)
    res = run_bass_kernel_spmd(nc, in_maps, core_ids=list(range(N_CORES)))
    return _assemble(res.results)


# revision 4
# speedup vs baseline: 1.0482x; 1.0482x over previous
"""Trainium2 Bass kernel for nn_ChannelAttentionLayer — v2.

Math (vs baseline kernel.py):
  - Conv biases cancel under batch-stats BN -> dropped.
  - Pad-ring keys: K_raw = V_raw = 0 there; K's BN offset shifts every
    query's scores equally (softmax-over-keys invariant), so ring keys get
    attention weight ~e^-70 -> dropped entirely.  Attention runs over the
    4096 interior keys = exactly 32 tiles of 128.
  - K needs NO elementwise normalization: softmax_k(Kbn^T Qbn) =
    softmax_k(Kraw^T Q^) with Q^ = (aq*ak) o Qraw + (cq*ak).  Only Q gets
    the affine; K raw conv output feeds the scores matmul directly.
  - V BN affine folded into the output epilogue (attn columns sum to 1).
  - Softmax denominator via an all-ones column appended to V^T.

Sharding: 8 cores = 4 batches x 2 query-halves.  Inputs per core:
  xq [128,CT,2244]  padded rows h*32..h*32+33 (Q conv input)
  xi [128,CT,4096]  interior keys: the OTHER half's 32 rows first (keys
                    0..2047, consumed early), own query rows last (keys
                    2048..4095, consumed late -> shipped late)
  wq [128,9,CT,256], wk/wv [128,CT,256], vecs [128,12]
BatchNorm statistics are combined with a tiny AllReduce (Q/K) and a second
one for V (deferred into the attention stream); V^T squares and part of
the K sum-of-squares run on the otherwise idle Pool engine, which also
takes a quarter of each output epilogue.
"""
import math

import numpy as np

import concourse.bass as bass
import concourse.mybir as mybir
import concourse.tile as tile
from concourse.bass_utils import run_bass_kernel_spmd

dt = mybir.dt
AF = mybir.ActivationFunctionType
ALU = mybir.AluOpType
F32 = dt.float32
F32R = dt.float32r

N_CORES = 8
CT = 2                   # channel tiles (256 = 2 x 128)
H = W = 64
HP = 66
NPOS = H * W             # 4096 interior positions (= keys per batch)
NPAD = HP * HP           # 4356 padded positions (reference BN count for K/V)
NKT = 32                 # key tiles: 4096 = 32 x 128
QSH = 2048               # query positions per core
QROWS = 34 * HP          # 2244: xq length
CSHIFT = 104.0           # softmax shift; global max score is ~101
EPS = 1e-5
NQ_TOT = float(4 * NPOS)
NKV_TOT2 = float(2 * 4 * NPAD)   # x2: both half-cores contribute full sums

# ---------------------------------------------------------------------------
# Workaround: this walrus build rejects >1 semaphore wait per instruction.
_UID = [0]


def _split_waits_in_module(nc):
    for fn in nc.m.functions:
        for blk in fn.blocks:
            insts = list(blk.instructions)
            if not any(
                i.sync_info and i.sync_info.on_wait and len(i.sync_info.on_wait) > 1
                for i in insts
            ):
                continue
            new = []
            for inst in insts:
                si = inst.sync_info
                waits = list(si.on_wait) if (si and si.on_wait) else []
                if len(waits) > 1:
                    for w in waits[:-1]:
                        _UID[0] += 1
                        new.append(
                            mybir.InstNoOp(
                                name=f"I-waitsplit-{_UID[0]}",
                                engine=inst.engine,
                                ins=[],
                                outs=[],
                                sync_info=mybir.SyncInfo(on_wait=[w], on_update=[]),
                            )
                        )
                    inst.sync_info = mybir.SyncInfo(
                        on_wait=waits[-1:], on_update=list(si.on_update or [])
                    )
                new.append(inst)
            del blk.instructions[:]
            for i in new:
                blk.instructions.append(i)


class TC(tile.TileContext):
    def __exit__(self, exc_type, exc_val, exc_tb):
        r = super().__exit__(exc_type, exc_val, exc_tb)
        if exc_type is None:
            _split_waits_in_module(self.nc)
        return r


# ---------------------------------------------------------------------------
def build_nc(reps: int = 1, skip_cc: bool = False):
    nc = bass.Bass("TRN2", target_bir_lowering=False, num_devices=N_CORES)

    xq_d = nc.dram_tensor("xq", [128, CT, QROWS], F32R, kind="ExternalInput")
    xi_d = nc.dram_tensor("xi", [128, CT, NPOS], F32R, kind="ExternalInput")
    wq_d = nc.dram_tensor("wq", [128, 9, CT, 256], F32R, kind="ExternalInput")
    wk_d = nc.dram_tensor("wk", [128, CT, 256], F32R, kind="ExternalInput")
    wv_d = nc.dram_tensor("wv", [128, CT, 256], F32R, kind="ExternalInput")
    vec_d = nc.dram_tensor("vecs", [128, 12], F32, kind="ExternalInput")
    y_d = nc.dram_tensor("y", [16, 128, 256], F32, kind="ExternalOutput")

    cc_in1 = nc.dram_tensor("cc_in1", [128, 8], F32)
    cc_out1 = nc.dram_tensor("cc_out1", [128, 8], F32, addr_space="Shared")
    cc_in2 = nc.dram_tensor("cc_in2", [128, 4], F32)
    cc_out2 = nc.dram_tensor("cc_out2", [128, 4], F32, addr_space="Shared")
    scr_ab = nc.dram_tensor("scr_ab", [512], F32)
    scr_v = nc.dram_tensor("scr_v", [512], F32)

    # Q conv row blocks: grid rows 1..32 grouped (7,7,7,7,4).
    QBLK = [(1, 7), (8, 7), (15, 7), (22, 7), (29, 4)]
    NKBLK = 4            # K conv: 4 blocks of 1024 interior positions per mt

    with TC(nc) as tc:
        with (
            tc.tile_pool(name="sb_in", bufs=1) as sb_in,
            tc.tile_pool(name="sb_w", bufs=1) as sb_w,
            tc.tile_pool(name="sb_small", bufs=1) as sb_small,
            tc.tile_pool(name="sb_tmp", bufs=3) as sb_tmp,
            tc.tile_pool(name="sb_e", bufs=5) as sb_e,
            tc.tile_pool(name="sb_out", bufs=4) as sb_out,
            tc.tile_pool(name="ps_s2", bufs=2, space="PSUM") as ps_s2,
            tc.tile_pool(name="psb1", bufs=4, space="PSUM") as psb1,
        ):
            def body(_it):
                f = F32

                # ------- loads: one bulk stream on the SP HWDGE queue,
                # ordered to match PE consumption: other-half keys (V segA +
                # K b0/b1) -> wq h0 + xq (Q conv) -> own-half keys (K b2/b3
                # + V segB, not needed until after Q mt0) -> wq h1.  Small
                # weights ride the Pool SWDGE queue concurrently.
                # (SWDGE InstTriggerDma doesn't lower inside For_i loops, so
                # the timing build falls back to the SP queue for these)
                wdma = nc.gpsimd if reps == 1 else nc.sync
                wv = sb_in.tile([128, CT, 256], F32R, tag="wv")
                wdma.dma_start(out=wv, in_=wv_d.ap())
                wk = sb_in.tile([128, CT, 256], F32R, tag="wk")
                wdma.dma_start(out=wk, in_=wk_d.ap())
                vecs = sb_in.tile([128, 12], f, tag="vecs")
                wdma.dma_start(out=vecs, in_=vec_d.ap())
                xi = sb_in.tile([128, CT, NPOS], F32R, tag="xi")
                xq = sb_in.tile([128, CT, QROWS], F32R, tag="xq")
                wq = sb_in.tile([128, 9, CT, 256], F32R, tag="wq")
                HQ = 17 * HP
                for lo, hi in ((0, 256), (256, 512), (512, 1024)):
                    for ci in range(CT):
                        nc.sync.dma_start(
                            out=xi[:, ci, lo:hi], in_=xi_d.ap()[:, ci, lo:hi]
                        )
                for ci in range(CT):
                    nc.sync.dma_start(
                        out=xq[:, ci, 0:HQ], in_=xq_d.ap()[:, ci, 0:HQ]
                    )
                nc.sync.dma_start(out=wq[:, :, :, 0:128], in_=wq_d.ap()[:, :, :, 0:128])
                for ci in range(CT):
                    nc.sync.dma_start(
                        out=xi[:, ci, 1024:2048], in_=xi_d.ap()[:, ci, 1024:2048]
                    )
                for ci in range(CT):
                    nc.sync.dma_start(
                        out=xq[:, ci, HQ:QROWS], in_=xq_d.ap()[:, ci, HQ:QROWS]
                    )
                for lo, hi in ((2048, 3072), (3072, 4096)):
                    for ci in range(CT):
                        nc.sync.dma_start(
                            out=xi[:, ci, lo:hi], in_=xi_d.ap()[:, ci, lo:hi]
                        )
                nc.sync.dma_start(out=wq[:, :, :, 128:256], in_=wq_d.ap()[:, :, :, 128:256])

                consts = sb_small.tile([128, 2], f, tag="consts")
                nc.vector.memset(consts[:, 0:1], EPS)
                nc.vector.memset(consts[:, 1:2], -CSHIFT)
                eps_t = consts[:, 0:1]
                negc_t = consts[:, 1:2]
                invn8 = sb_small.tile([128, 8], f, tag="invn8")
                nc.vector.memset(invn8[:, 0:2], 1.0 / NQ_TOT)
                nc.vector.memset(invn8[:, 2:4], 1.0 / NKV_TOT2)
                nc.vector.memset(invn8[:, 4:6], 1.0 / NQ_TOT)
                nc.vector.memset(invn8[:, 6:8], 1.0 / NKV_TOT2)

                qraw = sb_w.tile([128, CT, QSH], F32R, tag="qraw")
                kraw = sb_w.tile([128, CT, NPOS], F32R, tag="kraw")
                vt = sb_w.tile([128, NKT, 258], F32R, tag="vt")
                sums = sb_small.tile([128, 8], f, tag="sums")
                sums_v = sb_small.tile([128, 4], f, tag="sums_v")
                # per-block stat partials: Q sum 0:5, Q sumsq 5:10,
                # K sum 10:14, K sumsq 14:18
                qk_part = sb_small.tile([128, CT, 18], f, tag="qk_part")

                # ------- V^T conv (keys on partitions) + V stats -----------
                # vt[kt] rows = interior positions kt*128..kt*128+127.
                # V statistics: squares on the idle Pool engine, running
                # per-partition accumulators on DVE, and a single final
                # cross-partition ones-matmul pair (512 PE rows total).
                ones_r = sb_small.tile([128, 1], F32R, tag="ones_r")
                nc.vector.tensor_copy(
                    out=ones_r, in_=nc.const_aps.tensor(1.0, (128, 1), F32)
                )
                vsum_acc = sb_small.tile([128, 256], F32R, tag="vsum_acc")
                vsq_acc = sb_small.tile([128, 256], F32R, tag="vsq_acc")
                nc.vector.tensor_copy(
                    out=vsum_acc, in_=nc.const_aps.tensor(0.0, (128, 256), F32)
                )
                nc.vector.tensor_copy(
                    out=vsq_acc, in_=nc.const_aps.tensor(0.0, (128, 256), F32)
                )

                vstat_deferred = []

                def vt_tile(kt, defer=False):
                    pvt = psb1.tile([128, 258], f, tag="b1", name=f"pvt{kt}")
                    for ci in range(CT):
                        nc.tensor.matmul(
                            pvt[:, 0:256],
                            xi[:, ci, kt * 128:(kt + 1) * 128],
                            wv[:, ci, :],
                            start=(ci == 0), stop=(ci == CT - 1),
                        )
                    if 12 <= kt < 16:
                        nc.vector.tensor_copy(
                            out=vt[:, kt, 0:256], in_=pvt[:, 0:256]
                        )
                    else:
                        nc.scalar.activation(
                            out=vt[:, kt, 0:256], in_=pvt[:, 0:256],
                            func=AF.Copy,
                        )

                    def stats(kt=kt):
                        vt2 = sb_tmp.tile([128, 256], F32R, tag="vsq",
                                          name=f"vt2_{kt}", bufs=3)
                        nc.gpsimd.tensor_mul(
                            vt2, vt[:, kt, 0:256], vt[:, kt, 0:256]
                        )
                        nc.vector.tensor_add(vsum_acc, vsum_acc,
                                             vt[:, kt, 0:256])
                        nc.vector.tensor_add(vsq_acc, vsq_acc, vt2)

                    # ALL V-stat ops are deferred into the attention stream:
                    # squares on Pool, accumulator adds on DVE, both idle
                    # there, keeping every pre-attention engine queue clear.
                    vstat_deferred.append(stats)

                # ------- K conv (1x1 over interior) -> kraw stays RAW ------
                def emit_k_block(mt, bi):
                    sft = bi * 1024
                    pk = ps_s2.tile([128, 1024], f, tag="s2", name=f"pk{mt}{bi}")
                    for ci in range(CT):
                        for sub in range(0, 1024, 512):
                            nc.tensor.matmul(
                                pk[:, sub:sub + 512],
                                wk[:, ci, mt * 128:(mt + 1) * 128],
                                xi[:, ci, sft + sub:sft + sub + 512],
                                start=(ci == 0), stop=(ci == CT - 1),
                            )
                    nc.scalar.activation(
                        out=kraw[:, mt, sft:sft + 1024], in_=pk[:, 0:1024],
                        func=AF.Copy,
                        accum_out=qk_part[:, mt, 10 + bi:11 + bi],
                    )
                    # K sumsq on DVE (square + reduce): ACT carries the
                    # conv evictions and Pool the deferred V-stat squares
                    scr = sb_tmp.tile([128, 1024], f, tag="tmp",
                                      name=f"ksq{mt}_{bi}")
                    nc.vector.tensor_mul(
                        scr, kraw[:, mt, sft:sft + 1024],
                        kraw[:, mt, sft:sft + 1024]
                    )
                    nc.vector.reduce_sum(
                        out=qk_part[:, mt, 14 + bi:15 + bi], in_=scr,
                        axis=mybir.AxisListType.X,
                    )

                # ------- Q conv (3x3, 9 shifted spans; wrap garbage lands in
                # cols 64/65 of each 66-wide row, dropped at eviction).
                # Sum accumulates at eviction; Square pass right after.
                def emit_q_block(mt, r0, nr):
                    n = nr * HP - 2
                    pq = ps_s2.tile([128, 512], f, tag="s2", name=f"pq{mt}{r0}")
                    first = True
                    for tap in range(9):
                        ty, tx = tap // 3, tap % 3
                        sft = (r0 + ty - 1) * HP + tx
                        nc.tensor.matmul(
                            pq[:, 0:n],
                            wq[:, tap, 0, mt * 128:(mt + 1) * 128],
                            xq[:, 0, sft:sft + n],
                            start=first, stop=False,
                        )
                        nc.tensor.matmul(
                            pq[:, 0:n],
                            wq[:, tap, 1, mt * 128:(mt + 1) * 128],
                            xq[:, 1, sft:sft + n],
                            start=False, stop=(tap == 8),
                        )
                        first = False
                    bi = QBLK.index((r0, nr))
                    qsl = qraw[:, mt, (r0 - 1) * 64:(r0 - 1 + nr) * 64]
                    nc.scalar.activation(
                        out=qsl.rearrange("p (a b) -> p a b", a=nr),
                        in_=pq[:, 0:nr * HP]
                        .rearrange("p (a b) -> p a b", a=nr)[:, :, 0:64],
                        func=AF.Copy,
                        accum_out=qk_part[:, mt, bi:bi + 1],
                    )
                    scr = sb_tmp.tile([128, 512], f, tag="tmp",
                                      name=f"qsq{mt}_{r0}")
                    nc.scalar.activation(
                        out=scr[:, 0:nr * 64], in_=qsl, func=AF.Square,
                        accum_out=qk_part[:, mt, 5 + bi:6 + bi],
                    )

                # gathers: DVE free-axis reduces; emitted as soon as the
                # feeding partials are complete so only the last Q-mt1 pair
                # sits on the AllReduce critical path.
                def gather(ci, lo, hi, col):
                    nc.vector.reduce_sum(
                        out=sums[:, col:col + 1], in_=qk_part[:, ci, lo:hi],
                        axis=mybir.AxisListType.X,
                    )

                # ------- emission: V kt0-7 + K b0 (first xi chunks), then
                # Q mt0 ASAP (the big PE block, start ~10us), then V kt8-15
                # interleaved with K b1-b3, then Q mt1.
                for kt in range(8):
                    vt_tile(kt)
                for mt in range(2):
                    emit_k_block(mt, 0)
                for r0, nr in QBLK:
                    emit_q_block(0, r0, nr)
                gather(0, 0, 5, 0)
                gather(0, 5, 10, 4)
                for g in range(3):
                    if g < 2:
                        for kt in range(8 + 4 * g, 12 + 4 * g):
                            vt_tile(kt)
                    for mt in range(2):
                        emit_k_block(mt, g + 1)
                for ci in range(CT):
                    gather(ci, 10, 14, 2 + ci)
                    gather(ci, 14, 18, 6 + ci)
                for r0, nr in QBLK:
                    emit_q_block(1, r0, nr)
                gather(1, 0, 5, 1)
                gather(1, 5, 10, 5)

                # ---- AllReduce #1: Q/K stats (critical path) ----
                nc.sync.dma_start(out=cc_in1[:, :], in_=sums)
                sums_g = sb_small.tile([128, 8], f, tag="sums_g")
                if skip_cc:
                    nc.sync.dma_start(out=sums_g, in_=cc_in1[:, :])
                else:
                    nc.gpsimd.collective_compute(
                        "AllReduce", ALU.add,
                        replica_groups=[list(range(N_CORES))],
                        ins=[cc_in1.ap().opt()], outs=[cc_out1.ap().opt()],
                    )
                    nc.sync.dma_start(out=sums_g, in_=cc_out1[:, :])

                # ---- V^T tail: PE filler during the AR roundtrip ----
                for kt in range(16, NKT):
                    vt_tile(kt)
                nc.vector.tensor_copy(
                    out=vt[:, :, 256:257],
                    in_=nc.const_aps.tensor(1.0, (128, NKT, 1), F32),
                )
                nc.vector.tensor_copy(
                    out=vt[:, :, 257:258],
                    in_=nc.const_aps.tensor(0.0, (128, NKT, 1), F32),
                )

                # ---- V stats finale + AllReduce #2 + V affine/broadcast.
                # Emitted a few pairs INTO the attention stream so the PE
                # ones-matmuls never wait on the Pool/DVE accumulator chain,
                # which trails the segB convs by several us.  Only the
                # epilogue (~25us later) needs the result.
                avcv = sb_small.tile([128, 4], f, tag="avcv")
                avcv_b = sb_small.tile([128, 512], f, tag="avcv_b")
                av_b = avcv_b[:, 0:256]
                cv_b = avcv_b[:, 256:512]
                sums_vg = sb_small.tile([128, 4], f, tag="sums_vg")

                def emit_v_finale():
                    pvs = ps_s2.tile([1, 512], f, tag="s2", name="pvs")
                    nc.tensor.matmul(pvs[0:1, 0:256], ones_r, vsum_acc,
                                     start=True, stop=True)
                    nc.tensor.matmul(pvs[0:1, 256:512], ones_r, vsq_acc,
                                     start=True, stop=True)
                    vrow = sb_small.tile([1, 512], f, tag="vrow")
                    nc.vector.tensor_copy(out=vrow, in_=pvs)
                    nc.sync.dma_start(out=scr_v.ap(), in_=vrow)
                    nc.sync.dma_start(
                        out=sums_v,
                        in_=bass.AP(tensor=scr_v, offset=0,
                                    ap=[[1, 128], [256, 2], [128, 2]]),
                    )
                    nc.sync.dma_start(out=cc_in2[:, :], in_=sums_v)
                    if skip_cc:
                        nc.sync.dma_start(out=sums_vg, in_=cc_in2[:, :])
                    else:
                        nc.gpsimd.collective_compute(
                            "AllReduce", ALU.add,
                            replica_groups=[list(range(N_CORES))],
                            ins=[cc_in2.ap().opt()], outs=[cc_out2.ap().opt()],
                        )
                        nc.sync.dma_start(out=sums_vg, in_=cc_out2[:, :])
                    # V affine: av = gv*rsqrt(var+eps), cv = betav - av*mean
                    mv = sb_small.tile([128, 2], f, tag="mv")
                    vv = sb_small.tile([128, 2], f, tag="vv")
                    av = avcv[:, 0:2]
                    cv = avcv[:, 2:4]
                    nc.vector.tensor_scalar_mul(mv, sums_vg[:, 0:2],
                                                1.0 / NKV_TOT2)
                    nc.vector.tensor_scalar_mul(vv, sums_vg[:, 2:4],
                                                1.0 / NKV_TOT2)
                    nc.vector.tensor_mul(av, mv, mv)
                    nc.vector.tensor_sub(vv, vv, av)
                    nc.scalar.activation(out=vv, in_=vv, func=AF.Ln, bias=eps_t)
                    nc.scalar.activation(out=av, in_=vv, func=AF.Exp, scale=-0.5)
                    nc.vector.tensor_mul(av, vecs[:, 4:6], av)
                    nc.vector.tensor_mul(cv, av, mv)
                    nc.vector.tensor_sub(cv, vecs[:, 10:12], cv)
                    nc.sync.dma_start(
                        out=bass.AP(tensor=scr_ab, offset=0,
                                    ap=[[1, 128], [128, 4]]),
                        in_=avcv,
                    )
                    nc.sync.dma_start(
                        out=avcv_b,
                        in_=bass.AP(tensor=scr_ab, offset=0,
                                    ap=[[0, 128], [1, 512]]),
                    )

                # -------- Q/K affine, batched over 4 cols (q0,q1,k0,k1):
                # a = gamma * exp(-0.5*ln(var+eps)); c = beta - a*mean;
                # then fold K into Q: a' = aq*ak, c' = cq*ak.
                mm8 = sb_small.tile([128, 8], f, tag="mm8")
                var4 = sb_small.tile([128, 4], f, tag="var4")
                a4 = sb_small.tile([128, 4], f, tag="a4")
                cq2 = sb_small.tile([128, 2], f, tag="cq2")
                ap2 = sb_small.tile([128, 2], f, tag="ap2")
                cp2 = sb_small.tile([128, 2], f, tag="cp2")
                nc.vector.tensor_mul(mm8, sums_g, invn8)
                mean4 = mm8[:, 0:4]
                nc.vector.tensor_mul(var4, mean4, mean4)
                nc.vector.tensor_sub(var4, mm8[:, 4:8], var4)
                nc.scalar.activation(out=var4, in_=var4, func=AF.Ln, bias=eps_t)
                nc.scalar.activation(out=a4, in_=var4, func=AF.Exp, scale=-0.5)
                nc.vector.tensor_mul(a4, vecs[:, 0:4], a4)
                nc.vector.tensor_mul(cq2, a4[:, 0:2], mean4[:, 0:2])
                nc.vector.tensor_sub(cq2, vecs[:, 6:8], cq2)
                nc.vector.tensor_mul(ap2, a4[:, 0:2], a4[:, 2:4])
                nc.vector.tensor_mul(cp2, cq2, a4[:, 2:4])

                # ------- normalize Q in place (f32r): Q^ = a' o Q + c'.
                # Chunk 0 (the only one the first scores matmul needs) runs
                # on ACT right behind the affine's Ln/Exp; the rest and the
                # deferred segB V-stat ops follow on DVE.
                for j in range(4):
                    for ci in range(CT):
                        qsl = qraw[:, ci, j * 512:(j + 1) * 512]
                        if j == 0:
                            nc.scalar.activation(
                                out=qsl, in_=qsl, func=AF.Identity,
                                bias=cp2[:, ci:ci + 1], scale=ap2[:, ci:ci + 1],
                            )
                        else:
                            nc.vector.tensor_scalar(
                                qsl, qsl, ap2[:, ci:ci + 1], cp2[:, ci:ci + 1],
                                ALU.mult, ALU.add,
                            )

                # ---------------- attention ----------------
                # Flat software pipeline over (qb, kt-pair): the E->V matmuls
                # trail one pair behind so the exp latency stays off the PE
                # critical path, including across qb boundaries.
                PAIRS = [(2 * p, 2 * p + 2) for p in range(NKT // 2)]
                # last qb: finish with two single-kt pairs so the final
                # exp->out->epilogue chain into the drain is half as deep
                PAIRS_LAST = PAIRS[:-1] + [(30, 31), (31, 32)]
                po = {}

                def emit_out(qb, k0, k1, e2):
                    for kt in range(k0, k1):
                        off = (kt - k0) * 512
                        for qt in range(4):
                            nc.tensor.matmul(
                                po[qb][qt],
                                e2[:, off + qt * 128:off + (qt + 1) * 128],
                                vt[:, kt, :],
                                start=(kt == 0), stop=(kt == NKT - 1),
                            )

                def emit_epilogue(qb):
                    # recip on DVE; ACT scale-evicts (freeing PSUM fast);
                    # qt3's affine goes to Pool so the DVE chain and the
                    # last y DMA finish ~in parallel; y ships as two
                    # 2-tile DMAs (SP dispatch is 650ns a pop).
                    for pr in range(2):
                        ot2 = sb_out.tile([128, 2, 256], f, tag="ot",
                                          name=f"ot{qb}_{pr}", bufs=2)
                        for sub in range(2):
                            qt = pr * 2 + sub
                            qg = qb * 4 + qt
                            rd = sb_small.tile([128, 1], f, tag="rd",
                                               name=f"r{qg}", bufs=4)
                            nc.vector.reciprocal(out=rd,
                                                 in_=po[qb][qt][:, 256:257])
                            ot = ot2[:, sub, :]
                            nc.scalar.activation(
                                out=ot, in_=po[qb][qt][:, 0:256], func=AF.Copy,
                                scale=rd,
                            )
                            eng = nc.gpsimd if qt == 3 else nc.vector
                            eng.tensor_mul(ot, ot, av_b)
                            eng.tensor_add(ot, ot, cv_b)
                        nc.sync.dma_start(
                            out=bass.AP(
                                tensor=y_d,
                                offset=(qb * 4 + pr * 2) * 128 * 256,
                                ap=[[256, 128], [128 * 256, 2], [1, 256]],
                            ),
                            in_=ot2,
                        )

                pend = None
                for qb in range(4):
                    po[qb] = [
                        psb1.tile([128, 258], f, tag="b1", name=f"po{qb}_{i}")
                        for i in range(4)
                    ]
                    for (k0, k1) in (PAIRS_LAST if qb == 3 else PAIRS):
                        w = (k1 - k0) * 512
                        ps_s = ps_s2.tile([128, 1024], f, tag="s2",
                                          name=f"ps{qb}_{k0}")
                        for kt in range(k0, k1):
                            off = (kt - k0) * 512
                            for ci in range(CT):
                                nc.tensor.matmul(
                                    ps_s[:, off:off + 512],
                                    kraw[:, ci, kt * 128:(kt + 1) * 128],
                                    qraw[:, ci, qb * 512:(qb + 1) * 512],
                                    start=(ci == 0), stop=(ci == CT - 1),
                                )
                        e2 = sb_e.tile([128, 1024], F32R, tag="e",
                                       name=f"e{qb}_{k0}")
                        nc.scalar.activation(
                            out=e2[:, 0:w], in_=ps_s[:, 0:w], func=AF.Exp,
                            bias=negc_t,
                        )
                        if pend is not None:
                            emit_out(*pend)
                            if pend[2] == NKT:      # last pair of its qb
                                emit_epilogue(pend[0])
                        pend = (qb, k0, k1, e2)
                        if qb == 0 and k0 == 8:
                            # segB V-stat ops, deferred past the affine +
                            # normalize execution window
                            for stats in vstat_deferred:
                                stats()
                        if qb == 0 and k0 == 24:
                            emit_v_finale()
                emit_out(*pend)
                emit_epilogue(pend[0])

            if reps == 1:
                body(0)
            else:
                with tc.For_i(0, reps, 1) as it:
                    body(it)
    return nc


# ---------------------------------------------------------------------------
def _prep_inputs(x, Wq, Wk, Wv, gq, betaq, gk, betak, gv, betav):
    """Build the 8 per-core input maps (all fp32, pre-laid-out)."""
    x = np.asarray(x, np.float32)
    B = x.shape[0]
    xp_full = np.zeros((B, 256, HP, HP), np.float32)
    xp_full[:, :, 1:65, 1:65] = x

    wq_t = np.ascontiguousarray(
        np.asarray(Wq, np.float32).reshape(256, CT, 128, 3, 3)
        .transpose(2, 3, 4, 1, 0)
    ).reshape(128, 9, CT, 256)
    wk_t = np.ascontiguousarray(
        np.asarray(Wk, np.float32).reshape(256, CT, 128).transpose(2, 1, 0)
    )
    wv_t = np.ascontiguousarray(
        np.asarray(Wv, np.float32).reshape(256, CT, 128).transpose(2, 1, 0)
    )
    cols = [np.asarray(v, np.float32).reshape(CT, 128).T
            for v in (gq, gk, gv, betaq, betak, betav)]
    vecs = np.concatenate(cols, axis=1).astype(np.float32)  # (128, 12)
    vecs = np.ascontiguousarray(vecs)

    in_maps = []
    for core in range(N_CORES):
        b, h = core // 2, core % 2
        xq_b = np.ascontiguousarray(
            xp_full[b][:, h * 32:h * 32 + 34, :]
            .reshape(CT, 128, QROWS).transpose(1, 0, 2)
        )
        # keys 0:2048 = the OTHER half's 32 interior rows; keys 2048:4096 =
        # this core's own query rows (consumed late -> shipped late)
        oh = (1 - h) * 32
        xr = np.concatenate(
            [x[b][:, oh:oh + 32, :], x[b][:, h * 32:h * 32 + 32, :]], axis=1
        )
        xi_b = np.ascontiguousarray(
            xr.reshape(CT, 128, NPOS).transpose(1, 0, 2)
        )
        in_maps.append({
            "xq": xq_b, "xi": xi_b, "wq": wq_t, "wk": wk_t, "wv": wv_t,
            "vecs": vecs,
        })
    return in_maps


_NC_CACHE = {}


def _get_nc(reps=1, skip_cc=False):
    key = (reps, skip_cc)
    if key not in _NC_CACHE:
        _NC_CACHE[key] = build_nc(reps, skip_cc)
    return _NC_CACHE[key]


def _assemble(results):
    out = np.empty((4, 256, 4096), np.float32)
    for core, r in enumerate(results):
        b, h = core // 2, core % 2
        yc = r["y"].reshape(QSH, 256)          # (q, oc)
        out[b, :, h * QSH:(h + 1) * QSH] = yc.T
    return out.reshape(4, 256, 64, 64)


def kernel(x, Wq, bq, gq, betaq, Wk, bk, gk, betak, Wv, bv, gv, betav,
           _reps=1):
    # bq/bk/bv are mathematically irrelevant: BatchNorm with batch statistics
    # removes any per-channel constant shift (including the pad-ring bias).
    in_maps = _prep_inputs(x, Wq, Wk, Wv, gq, betaq, gk, betak, gv, betav)
    nc = _get_nc(_reps)
    res = run_bass_kernel_spmd(nc, in_maps, core_ids=list(range(N_CORES)))
    return _assemble(res.results)


# revision 5
# speedup vs baseline: 1.0556x; 1.0071x over previous
"""Trainium2 Bass kernel for nn_ChannelAttentionLayer — v2.

Math (vs baseline kernel.py):
  - Conv biases cancel under batch-stats BN -> dropped.
  - Pad-ring keys: K_raw = V_raw = 0 there; K's BN offset shifts every
    query's scores equally (softmax-over-keys invariant), so ring keys get
    attention weight ~e^-70 -> dropped entirely.  Attention runs over the
    4096 interior keys = exactly 32 tiles of 128.
  - K needs NO elementwise normalization: softmax_k(Kbn^T Qbn) =
    softmax_k(Kraw^T Q^) with Q^ = (aq*ak) o Qraw + (cq*ak).  Only Q gets
    the affine; K raw conv output feeds the scores matmul directly.
  - V BN affine folded into the output epilogue (attn columns sum to 1).
  - Softmax denominator via an all-ones column appended to V^T.

Sharding: 8 cores = 4 batches x 2 query-halves.  Inputs per core:
  xq [128,CT,2244]  padded rows h*32..h*32+33 (Q conv input)
  xi [128,CT,4096]  interior keys: the OTHER half's 32 rows first (keys
                    0..2047, consumed early), own query rows last (keys
                    2048..4095, consumed late -> shipped late)
  wq [128,9,CT,256], wk/wv [128,CT,256], vecs [128,12]
BatchNorm statistics are combined with a tiny AllReduce (Q/K) and a second
one for V (deferred into the attention stream); V^T squares and part of
the K sum-of-squares run on the otherwise idle Pool engine, which also
takes a quarter of each output epilogue.
"""
import math

import numpy as np

import concourse.bass as bass
import concourse.mybir as mybir
import concourse.tile as tile
from concourse.bass_utils import run_bass_kernel_spmd

dt = mybir.dt
AF = mybir.ActivationFunctionType
ALU = mybir.AluOpType
F32 = dt.float32
F32R = dt.float32r

N_CORES = 8
CT = 2                   # channel tiles (256 = 2 x 128)
H = W = 64
HP = 66
NPOS = H * W             # 4096 interior positions (= keys per batch)
NPAD = HP * HP           # 4356 padded positions (reference BN count for K/V)
NKT = 32                 # key tiles: 4096 = 32 x 128
QSH = 2048               # query positions per core
QROWS = 34 * HP          # 2244: xq length
CSHIFT = 104.0           # softmax shift; global max score is ~101
EPS = 1e-5
NQ_TOT = float(4 * NPOS)
NKV_TOT2 = float(2 * 4 * NPAD)   # x2: both half-cores contribute full sums

# ---------------------------------------------------------------------------
# Workaround: this walrus build rejects >1 semaphore wait per instruction.
_UID = [0]


def _split_waits_in_module(nc):
    for fn in nc.m.functions:
        for blk in fn.blocks:
            insts = list(blk.instructions)
            if not any(
                i.sync_info and i.sync_info.on_wait and len(i.sync_info.on_wait) > 1
                for i in insts
            ):
                continue
            new = []
            for inst in insts:
                si = inst.sync_info
                waits = list(si.on_wait) if (si and si.on_wait) else []
                if len(waits) > 1:
                    for w in waits[:-1]:
                        _UID[0] += 1
                        new.append(
                            mybir.InstNoOp(
                                name=f"I-waitsplit-{_UID[0]}",
                                engine=inst.engine,
                                ins=[],
                                outs=[],
                                sync_info=mybir.SyncInfo(on_wait=[w], on_update=[]),
                            )
                        )
                    inst.sync_info = mybir.SyncInfo(
                        on_wait=waits[-1:], on_update=list(si.on_update or [])
                    )
                new.append(inst)
            del blk.instructions[:]
            for i in new:
                blk.instructions.append(i)


class TC(tile.TileContext):
    def __exit__(self, exc_type, exc_val, exc_tb):
        r = super().__exit__(exc_type, exc_val, exc_tb)
        if exc_type is None:
            _split_waits_in_module(self.nc)
        return r


# ---------------------------------------------------------------------------
def build_nc(reps: int = 1, skip_cc: bool = False):
    nc = bass.Bass("TRN2", target_bir_lowering=False, num_devices=N_CORES)

    xq_d = nc.dram_tensor("xq", [128, CT, QROWS], F32R, kind="ExternalInput")
    xi_d = nc.dram_tensor("xi", [128, CT, NPOS], F32R, kind="ExternalInput")
    wq_d = nc.dram_tensor("wq", [128, 9, CT, 256], F32R, kind="ExternalInput")
    wk_d = nc.dram_tensor("wk", [128, CT, 256], F32R, kind="ExternalInput")
    wv_d = nc.dram_tensor("wv", [128, CT, 256], F32R, kind="ExternalInput")
    vec_d = nc.dram_tensor("vecs", [128, 12], F32, kind="ExternalInput")
    y_d = nc.dram_tensor("y", [16, 128, 256], F32, kind="ExternalOutput")

    cc_in1 = nc.dram_tensor("cc_in1", [128, 8], F32)
    cc_out1 = nc.dram_tensor("cc_out1", [128, 8], F32, addr_space="Shared")
    cc_in2 = nc.dram_tensor("cc_in2", [128, 4], F32)
    cc_out2 = nc.dram_tensor("cc_out2", [128, 4], F32, addr_space="Shared")
    scr_ab = nc.dram_tensor("scr_ab", [512], F32)
    scr_v = nc.dram_tensor("scr_v", [512], F32)

    # Q conv row blocks: grid rows 1..32 grouped (7,7,7,7,4).
    QBLK = [(1, 7), (8, 7), (15, 7), (22, 7), (29, 4)]
    NKBLK = 4            # K conv: 4 blocks of 1024 interior positions per mt

    with TC(nc) as tc:
        with (
            tc.tile_pool(name="sb_in", bufs=1) as sb_in,
            tc.tile_pool(name="sb_w", bufs=1) as sb_w,
            tc.tile_pool(name="sb_small", bufs=1) as sb_small,
            tc.tile_pool(name="sb_tmp", bufs=3) as sb_tmp,
            tc.tile_pool(name="sb_e", bufs=5) as sb_e,
            tc.tile_pool(name="sb_out", bufs=4) as sb_out,
            tc.tile_pool(name="ps_s2", bufs=2, space="PSUM") as ps_s2,
            tc.tile_pool(name="psb1", bufs=4, space="PSUM") as psb1,
        ):
            def body(_it):
                f = F32

                # ------- loads: one bulk stream on the SP HWDGE queue,
                # ordered to match PE consumption: other-half keys (V segA +
                # K b0/b1) -> wq h0 + xq (Q conv) -> own-half keys (K b2/b3
                # + V segB, not needed until after Q mt0) -> wq h1.  Small
                # weights ride the Pool SWDGE queue concurrently.
                # (SWDGE InstTriggerDma doesn't lower inside For_i loops, so
                # the timing build falls back to the SP queue for these)
                wdma = nc.gpsimd if reps == 1 else nc.sync
                wv = sb_in.tile([128, CT, 256], F32R, tag="wv")
                wdma.dma_start(out=wv, in_=wv_d.ap())
                wk = sb_in.tile([128, CT, 256], F32R, tag="wk")
                wdma.dma_start(out=wk, in_=wk_d.ap())
                vecs = sb_in.tile([128, 12], f, tag="vecs")
                wdma.dma_start(out=vecs, in_=vec_d.ap())
                xi = sb_in.tile([128, CT, NPOS], F32R, tag="xi")
                xq = sb_in.tile([128, CT, QROWS], F32R, tag="xq")
                wq = sb_in.tile([128, 9, CT, 256], F32R, tag="wq")
                HQ = 17 * HP
                R9 = 9 * HP
                for lo, hi in ((0, 256), (256, 512), (512, 1024)):
                    for ci in range(CT):
                        nc.sync.dma_start(
                            out=xi[:, ci, lo:hi], in_=xi_d.ap()[:, ci, lo:hi]
                        )
                for ci in range(CT):
                    nc.sync.dma_start(
                        out=xq[:, ci, 0:R9], in_=xq_d.ap()[:, ci, 0:R9]
                    )
                nc.sync.dma_start(out=wq[:, :, :, 0:128], in_=wq_d.ap()[:, :, :, 0:128])
                for ci in range(CT):
                    nc.sync.dma_start(
                        out=xq[:, ci, R9:HQ], in_=xq_d.ap()[:, ci, R9:HQ]
                    )
                for ci in range(CT):
                    nc.sync.dma_start(
                        out=xi[:, ci, 1024:2048], in_=xi_d.ap()[:, ci, 1024:2048]
                    )
                for ci in range(CT):
                    nc.sync.dma_start(
                        out=xq[:, ci, HQ:QROWS], in_=xq_d.ap()[:, ci, HQ:QROWS]
                    )
                for lo, hi in ((2048, 3072), (3072, 4096)):
                    for ci in range(CT):
                        nc.sync.dma_start(
                            out=xi[:, ci, lo:hi], in_=xi_d.ap()[:, ci, lo:hi]
                        )
                nc.sync.dma_start(out=wq[:, :, :, 128:256], in_=wq_d.ap()[:, :, :, 128:256])

                consts = sb_small.tile([128, 2], f, tag="consts")
                nc.vector.memset(consts[:, 0:1], EPS)
                nc.vector.memset(consts[:, 1:2], -CSHIFT)
                eps_t = consts[:, 0:1]
                negc_t = consts[:, 1:2]
                invn8 = sb_small.tile([128, 8], f, tag="invn8")
                nc.vector.memset(invn8[:, 0:2], 1.0 / NQ_TOT)
                nc.vector.memset(invn8[:, 2:4], 1.0 / NKV_TOT2)
                nc.vector.memset(invn8[:, 4:6], 1.0 / NQ_TOT)
                nc.vector.memset(invn8[:, 6:8], 1.0 / NKV_TOT2)

                qraw = sb_w.tile([128, CT, QSH], F32R, tag="qraw")
                kraw = sb_w.tile([128, CT, NPOS], F32R, tag="kraw")
                vt = sb_w.tile([128, NKT, 258], F32R, tag="vt")
                sums = sb_small.tile([128, 8], f, tag="sums")
                sums_v = sb_small.tile([128, 4], f, tag="sums_v")
                # per-block stat partials: Q sum 0:5, Q sumsq 5:10,
                # K sum 10:14, K sumsq 14:18
                qk_part = sb_small.tile([128, CT, 18], f, tag="qk_part")

                # ------- V^T conv (keys on partitions) + V stats -----------
                # vt[kt] rows = interior positions kt*128..kt*128+127.
                # V statistics: squares on the idle Pool engine, running
                # per-partition accumulators on DVE, and a single final
                # cross-partition ones-matmul pair (512 PE rows total).
                ones_r = sb_small.tile([128, 1], F32R, tag="ones_r")
                nc.vector.tensor_copy(
                    out=ones_r, in_=nc.const_aps.tensor(1.0, (128, 1), F32)
                )
                vsum_acc = sb_small.tile([128, 256], F32R, tag="vsum_acc")
                vsq_acc = sb_small.tile([128, 256], F32R, tag="vsq_acc")
                nc.vector.tensor_copy(
                    out=vsum_acc, in_=nc.const_aps.tensor(0.0, (128, 256), F32)
                )
                nc.vector.tensor_copy(
                    out=vsq_acc, in_=nc.const_aps.tensor(0.0, (128, 256), F32)
                )

                vstat_deferred = []

                def vt_tile(kt, defer=False):
                    pvt = psb1.tile([128, 258], f, tag="b1", name=f"pvt{kt}")
                    for ci in range(CT):
                        nc.tensor.matmul(
                            pvt[:, 0:256],
                            xi[:, ci, kt * 128:(kt + 1) * 128],
                            wv[:, ci, :],
                            start=(ci == 0), stop=(ci == CT - 1),
                        )
                    if 12 <= kt < 16:
                        nc.vector.tensor_copy(
                            out=vt[:, kt, 0:256], in_=pvt[:, 0:256]
                        )
                    else:
                        nc.scalar.activation(
                            out=vt[:, kt, 0:256], in_=pvt[:, 0:256],
                            func=AF.Copy,
                        )

                    def stats(kt=kt):
                        vt2 = sb_tmp.tile([128, 256], F32R, tag="vsq",
                                          name=f"vt2_{kt}", bufs=3)
                        nc.gpsimd.tensor_mul(
                            vt2, vt[:, kt, 0:256], vt[:, kt, 0:256]
                        )
                        nc.vector.tensor_add(vsum_acc, vsum_acc,
                                             vt[:, kt, 0:256])
                        nc.vector.tensor_add(vsq_acc, vsq_acc, vt2)

                    # ALL V-stat ops are deferred into the attention stream:
                    # squares on Pool, accumulator adds on DVE, both idle
                    # there, keeping every pre-attention engine queue clear.
                    vstat_deferred.append(stats)

                # ------- K conv (1x1 over interior) -> kraw stays RAW ------
                def emit_k_block(mt, bi):
                    sft = bi * 1024
                    pk = ps_s2.tile([128, 1024], f, tag="s2", name=f"pk{mt}{bi}")
                    for ci in range(CT):
                        for sub in range(0, 1024, 512):
                            nc.tensor.matmul(
                                pk[:, sub:sub + 512],
                                wk[:, ci, mt * 128:(mt + 1) * 128],
                                xi[:, ci, sft + sub:sft + sub + 512],
                                start=(ci == 0), stop=(ci == CT - 1),
                            )
                    nc.scalar.activation(
                        out=kraw[:, mt, sft:sft + 1024], in_=pk[:, 0:1024],
                        func=AF.Copy,
                        accum_out=qk_part[:, mt, 10 + bi:11 + bi],
                    )
                    # K sumsq on DVE (square + reduce): ACT carries the
                    # conv evictions and Pool the deferred V-stat squares
                    scr = sb_tmp.tile([128, 1024], f, tag="tmp",
                                      name=f"ksq{mt}_{bi}")
                    nc.vector.tensor_mul(
                        scr, kraw[:, mt, sft:sft + 1024],
                        kraw[:, mt, sft:sft + 1024]
                    )
                    nc.vector.reduce_sum(
                        out=qk_part[:, mt, 14 + bi:15 + bi], in_=scr,
                        axis=mybir.AxisListType.X,
                    )

                # ------- Q conv (3x3, 9 shifted spans; wrap garbage lands in
                # cols 64/65 of each 66-wide row, dropped at eviction).
                # Sum accumulates at eviction; Square pass right after.
                def emit_q_block(mt, r0, nr):
                    n = nr * HP - 2
                    pq = ps_s2.tile([128, 512], f, tag="s2", name=f"pq{mt}{r0}")
                    first = True
                    for tap in range(9):
                        ty, tx = tap // 3, tap % 3
                        sft = (r0 + ty - 1) * HP + tx
                        nc.tensor.matmul(
                            pq[:, 0:n],
                            wq[:, tap, 0, mt * 128:(mt + 1) * 128],
                            xq[:, 0, sft:sft + n],
                            start=first, stop=False,
                        )
                        nc.tensor.matmul(
                            pq[:, 0:n],
                            wq[:, tap, 1, mt * 128:(mt + 1) * 128],
                            xq[:, 1, sft:sft + n],
                            start=False, stop=(tap == 8),
                        )
                        first = False
                    bi = QBLK.index((r0, nr))
                    qsl = qraw[:, mt, (r0 - 1) * 64:(r0 - 1 + nr) * 64]
                    nc.scalar.activation(
                        out=qsl.rearrange("p (a b) -> p a b", a=nr),
                        in_=pq[:, 0:nr * HP]
                        .rearrange("p (a b) -> p a b", a=nr)[:, :, 0:64],
                        func=AF.Copy,
                        accum_out=qk_part[:, mt, bi:bi + 1],
                    )
                    scr = sb_tmp.tile([128, 512], f, tag="tmp",
                                      name=f"qsq{mt}_{r0}")
                    nc.scalar.activation(
                        out=scr[:, 0:nr * 64], in_=qsl, func=AF.Square,
                        accum_out=qk_part[:, mt, 5 + bi:6 + bi],
                    )

                # gathers: DVE free-axis reduces; emitted as soon as the
                # feeding partials are complete so only the last Q-mt1 pair
                # sits on the AllReduce critical path.
                def gather(ci, lo, hi, col):
                    nc.vector.reduce_sum(
                        out=sums[:, col:col + 1], in_=qk_part[:, ci, lo:hi],
                        axis=mybir.AxisListType.X,
                    )

                # ------- emission: V kt0-7 + K b0 (first xi chunks), then
                # Q mt0 ASAP (the big PE block, start ~10us), then V kt8-15
                # interleaved with K b1-b3, then Q mt1.
                for kt in range(8):
                    vt_tile(kt)
                for mt in range(2):
                    emit_k_block(mt, 0)
                for r0, nr in QBLK:
                    emit_q_block(0, r0, nr)
                gather(0, 0, 5, 0)
                gather(0, 5, 10, 4)
                for g in range(3):
                    if g < 1:
                        for kt in range(8 + 4 * g, 12 + 4 * g):
                            vt_tile(kt)
                    for mt in range(2):
                        emit_k_block(mt, g + 1)
                for ci in range(CT):
                    gather(ci, 10, 14, 2 + ci)
                    gather(ci, 14, 18, 6 + ci)
                for r0, nr in QBLK:
                    emit_q_block(1, r0, nr)
                gather(1, 0, 5, 1)
                gather(1, 5, 10, 5)

                # ---- AllReduce #1: Q/K stats (critical path) ----
                nc.sync.dma_start(out=cc_in1[:, :], in_=sums)
                sums_g = sb_small.tile([128, 8], f, tag="sums_g")
                if skip_cc:
                    nc.sync.dma_start(out=sums_g, in_=cc_in1[:, :])
                else:
                    nc.gpsimd.collective_compute(
                        "AllReduce", ALU.add,
                        replica_groups=[list(range(N_CORES))],
                        ins=[cc_in1.ap().opt()], outs=[cc_out1.ap().opt()],
                    )
                    nc.sync.dma_start(out=sums_g, in_=cc_out1[:, :])

                # ---- V^T tail: PE filler during the AR roundtrip ----
                for kt in range(12, NKT):
                    vt_tile(kt)
                nc.vector.tensor_copy(
                    out=vt[:, :, 256:257],
                    in_=nc.const_aps.tensor(1.0, (128, NKT, 1), F32),
                )
                nc.vector.tensor_copy(
                    out=vt[:, :, 257:258],
                    in_=nc.const_aps.tensor(0.0, (128, NKT, 1), F32),
                )

                # ---- V stats finale + AllReduce #2 + V affine/broadcast.
                # Emitted a few pairs INTO the attention stream so the PE
                # ones-matmuls never wait on the Pool/DVE accumulator chain,
                # which trails the segB convs by several us.  Only the
                # epilogue (~25us later) needs the result.
                avcv = sb_small.tile([128, 4], f, tag="avcv")
                avcv_b = sb_small.tile([128, 512], f, tag="avcv_b")
                av_b = avcv_b[:, 0:256]
                cv_b = avcv_b[:, 256:512]
                sums_vg = sb_small.tile([128, 4], f, tag="sums_vg")

                def emit_v_finale():
                    pvs = ps_s2.tile([1, 512], f, tag="s2", name="pvs")
                    nc.tensor.matmul(pvs[0:1, 0:256], ones_r, vsum_acc,
                                     start=True, stop=True)
                    nc.tensor.matmul(pvs[0:1, 256:512], ones_r, vsq_acc,
                                     start=True, stop=True)
                    vrow = sb_small.tile([1, 512], f, tag="vrow")
                    nc.vector.tensor_copy(out=vrow, in_=pvs)
                    nc.sync.dma_start(out=scr_v.ap(), in_=vrow)
                    nc.sync.dma_start(
                        out=sums_v,
                        in_=bass.AP(tensor=scr_v, offset=0,
                                    ap=[[1, 128], [256, 2], [128, 2]]),
                    )
                    nc.sync.dma_start(out=cc_in2[:, :], in_=sums_v)
                    if skip_cc:
                        nc.sync.dma_start(out=sums_vg, in_=cc_in2[:, :])
                    else:
                        nc.gpsimd.collective_compute(
                            "AllReduce", ALU.add,
                            replica_groups=[list(range(N_CORES))],
                            ins=[cc_in2.ap().opt()], outs=[cc_out2.ap().opt()],
                        )
                        nc.sync.dma_start(out=sums_vg, in_=cc_out2[:, :])
                    # V affine: av = gv*rsqrt(var+eps), cv = betav - av*mean
                    mv = sb_small.tile([128, 2], f, tag="mv")
                    vv = sb_small.tile([128, 2], f, tag="vv")
                    av = avcv[:, 0:2]
                    cv = avcv[:, 2:4]
                    nc.vector.tensor_scalar_mul(mv, sums_vg[:, 0:2],
                                                1.0 / NKV_TOT2)
                    nc.vector.tensor_scalar_mul(vv, sums_vg[:, 2:4],
                                                1.0 / NKV_TOT2)
                    nc.vector.tensor_mul(av, mv, mv)
                    nc.vector.tensor_sub(vv, vv, av)
                    nc.scalar.activation(out=vv, in_=vv, func=AF.Ln, bias=eps_t)
                    nc.scalar.activation(out=av, in_=vv, func=AF.Exp, scale=-0.5)
                    nc.vector.tensor_mul(av, vecs[:, 4:6], av)
                    nc.vector.tensor_mul(cv, av, mv)
                    nc.vector.tensor_sub(cv, vecs[:, 10:12], cv)
                    nc.sync.dma_start(
                        out=bass.AP(tensor=scr_ab, offset=0,
                                    ap=[[1, 128], [128, 4]]),
                        in_=avcv,
                    )
                    nc.sync.dma_start(
                        out=avcv_b,
                        in_=bass.AP(tensor=scr_ab, offset=0,
                                    ap=[[0, 128], [1, 512]]),
                    )

                # -------- Q/K affine, batched over 4 cols (q0,q1,k0,k1):
                # a = gamma * exp(-0.5*ln(var+eps)); c = beta - a*mean;
                # then fold K into Q: a' = aq*ak, c' = cq*ak.
                mm8 = sb_small.tile([128, 8], f, tag="mm8")
                var4 = sb_small.tile([128, 4], f, tag="var4")
                a4 = sb_small.tile([128, 4], f, tag="a4")
                cq2 = sb_small.tile([128, 2], f, tag="cq2")
                ap2 = sb_small.tile([128, 2], f, tag="ap2")
                cp2 = sb_small.tile([128, 2], f, tag="cp2")
                nc.vector.tensor_mul(mm8, sums_g, invn8)
                mean4 = mm8[:, 0:4]
                nc.vector.tensor_mul(var4, mean4, mean4)
                nc.vector.tensor_sub(var4, mm8[:, 4:8], var4)
                nc.scalar.activation(out=var4, in_=var4, func=AF.Ln, bias=eps_t)
                nc.scalar.activation(out=a4, in_=var4, func=AF.Exp, scale=-0.5)
                nc.vector.tensor_mul(a4, vecs[:, 0:4], a4)
                nc.vector.tensor_mul(cq2, a4[:, 0:2], mean4[:, 0:2])
                nc.vector.tensor_sub(cq2, vecs[:, 6:8], cq2)
                nc.vector.tensor_mul(ap2, a4[:, 0:2], a4[:, 2:4])
                nc.vector.tensor_mul(cp2, cq2, a4[:, 2:4])

                # ------- normalize Q in place (f32r): Q^ = a' o Q + c'.
                # Chunk 0 (the only one the first scores matmul needs) runs
                # on ACT right behind the affine's Ln/Exp; the rest and the
                # deferred segB V-stat ops follow on DVE.
                for j in range(4):
                    for ci in range(CT):
                        qsl = qraw[:, ci, j * 512:(j + 1) * 512]
                        if j == 0:
                            nc.scalar.activation(
                                out=qsl, in_=qsl, func=AF.Identity,
                                bias=cp2[:, ci:ci + 1], scale=ap2[:, ci:ci + 1],
                            )
                        else:
                            nc.vector.tensor_scalar(
                                qsl, qsl, ap2[:, ci:ci + 1], cp2[:, ci:ci + 1],
                                ALU.mult, ALU.add,
                            )

                # ---------------- attention ----------------
                # Flat software pipeline over (qb, kt-pair): the E->V matmuls
                # trail one pair behind so the exp latency stays off the PE
                # critical path, including across qb boundaries.
                PAIRS = [(2 * p, 2 * p + 2) for p in range(NKT // 2)]
                # last qb: finish with two single-kt pairs so the final
                # exp->out->epilogue chain into the drain is half as deep
                PAIRS_LAST = PAIRS[:-1] + [(30, 31), (31, 32)]
                po = {}

                def emit_out(qb, k0, k1, e2):
                    for kt in range(k0, k1):
                        off = (kt - k0) * 512
                        for qt in range(4):
                            nc.tensor.matmul(
                                po[qb][qt],
                                e2[:, off + qt * 128:off + (qt + 1) * 128],
                                vt[:, kt, :],
                                start=(kt == 0), stop=(kt == NKT - 1),
                            )

                def emit_epilogue(qb):
                    # recip on DVE; ACT scale-evicts (freeing PSUM fast);
                    # qt3's affine goes to Pool so the DVE chain and the
                    # last y DMA finish ~in parallel; y ships as two
                    # 2-tile DMAs (SP dispatch is 650ns a pop).
                    for pr in range(2):
                        ot2 = sb_out.tile([128, 2, 256], f, tag="ot",
                                          name=f"ot{qb}_{pr}", bufs=2)
                        for sub in range(2):
                            qt = pr * 2 + sub
                            qg = qb * 4 + qt
                            rd = sb_small.tile([128, 1], f, tag="rd",
                                               name=f"r{qg}", bufs=4)
                            nc.vector.reciprocal(out=rd,
                                                 in_=po[qb][qt][:, 256:257])
                            ot = ot2[:, sub, :]
                            nc.scalar.activation(
                                out=ot, in_=po[qb][qt][:, 0:256], func=AF.Copy,
                                scale=rd,
                            )
                            eng = nc.gpsimd if qt == 3 else nc.vector
                            eng.tensor_mul(ot, ot, av_b)
                            eng.tensor_add(ot, ot, cv_b)
                        nc.sync.dma_start(
                            out=bass.AP(
                                tensor=y_d,
                                offset=(qb * 4 + pr * 2) * 128 * 256,
                                ap=[[256, 128], [128 * 256, 2], [1, 256]],
                            ),
                            in_=ot2,
                        )

                pend = None
                for qb in range(4):
                    po[qb] = [
                        psb1.tile([128, 258], f, tag="b1", name=f"po{qb}_{i}")
                        for i in range(4)
                    ]
                    for (k0, k1) in (PAIRS_LAST if qb == 3 else PAIRS):
                        w = (k1 - k0) * 512
                        ps_s = ps_s2.tile([128, 1024], f, tag="s2",
                                          name=f"ps{qb}_{k0}")
                        for kt in range(k0, k1):
                            off = (kt - k0) * 512
                            for ci in range(CT):
                                nc.tensor.matmul(
                                    ps_s[:, off:off + 512],
                                    kraw[:, ci, kt * 128:(kt + 1) * 128],
                                    qraw[:, ci, qb * 512:(qb + 1) * 512],
                                    start=(ci == 0), stop=(ci == CT - 1),
                                )
                        e2 = sb_e.tile([128, 1024], F32R, tag="e",
                                       name=f"e{qb}_{k0}")
                        nc.scalar.activation(
                            out=e2[:, 0:w], in_=ps_s[:, 0:w], func=AF.Exp,
                            bias=negc_t,
                        )
                        if pend is not None:
                            emit_out(*pend)
                            if pend[2] == NKT:      # last pair of its qb
                                emit_epilogue(pend[0])
                        pend = (qb, k0, k1, e2)
                        if qb == 0 and k0 == 8:
                            # segB V-stat ops, deferred past the affine +
                            # normalize execution window
                            for stats in vstat_deferred:
                                stats()
                        if qb == 0 and k0 == 24:
                            emit_v_finale()
                emit_out(*pend)
                emit_epilogue(pend[0])

            if reps == 1:
                body(0)
            else:
                with tc.For_i(0, reps, 1) as it:
                    body(it)
    return nc


# ---------------------------------------------------------------------------
def _prep_inputs(x, Wq, Wk, Wv, gq, betaq, gk, betak, gv, betav):
    """Build the 8 per-core input maps (all fp32, pre-laid-out)."""
    x = np.asarray(x, np.float32)
    B = x.shape[0]
    xp_full = np.zeros((B, 256, HP, HP), np.float32)
    xp_full[:, :, 1:65, 1:65] = x

    wq_t = np.ascontiguousarray(
        np.asarray(Wq, np.float32).reshape(256, CT, 128, 3, 3)
        .transpose(2, 3, 4, 1, 0)
    ).reshape(128, 9, CT, 256)
    wk_t = np.ascontiguousarray(
        np.asarray(Wk, np.float32).reshape(256, CT, 128).transpose(2, 1, 0)
    )
    wv_t = np.ascontiguousarray(
        np.asarray(Wv, np.float32).reshape(256, CT, 128).transpose(2, 1, 0)
    )
    cols = [np.asarray(v, np.float32).reshape(CT, 128).T
            for v in (gq, gk, gv, betaq, betak, betav)]
    vecs = np.concatenate(cols, axis=1).astype(np.float32)  # (128, 12)
    vecs = np.ascontiguousarray(vecs)

    in_maps = []
    for core in range(N_CORES):
        b, h = core // 2, core % 2
        xq_b = np.ascontiguousarray(
            xp_full[b][:, h * 32:h * 32 + 34, :]
            .reshape(CT, 128, QROWS).transpose(1, 0, 2)
        )
        # keys 0:2048 = the OTHER half's 32 interior rows; keys 2048:4096 =
        # this core's own query rows (consumed late -> shipped late)
        oh = (1 - h) * 32
        xr = np.concatenate(
            [x[b][:, oh:oh + 32, :], x[b][:, h * 32:h * 32 + 32, :]], axis=1
        )
        xi_b = np.ascontiguousarray(
            xr.reshape(CT, 128, NPOS).transpose(1, 0, 2)
        )
        in_maps.append({
            "xq": xq_b, "xi": xi_b, "wq": wq_t, "wk": wk_t, "wv": wv_t,
            "vecs": vecs,
        })
    return in_maps


_NC_CACHE = {}


def _get_nc(reps=1, skip_cc=False):
    key = (reps, skip_cc)
    if key not in _NC_CACHE:
        _NC_CACHE[key] = build_nc(reps, skip_cc)
    return _NC_CACHE[key]


def _assemble(results):
    out = np.empty((4, 256, 4096), np.float32)
    for core, r in enumerate(results):
        b, h = core // 2, core % 2
        yc = r["y"].reshape(QSH, 256)          # (q, oc)
        out[b, :, h * QSH:(h + 1) * QSH] = yc.T
    return out.reshape(4, 256, 64, 64)


def kernel(x, Wq, bq, gq, betaq, Wk, bk, gk, betak, Wv, bv, gv, betav,
           _reps=1):
    # bq/bk/bv are mathematically irrelevant: BatchNorm with batch statistics
    # removes any per-channel constant shift (including the pad-ring bias).
    in_maps = _prep_inputs(x, Wq, Wk, Wv, gq, betaq, gk, betak, gv, betav)
    nc = _get_nc(_reps)
    res = run_bass_kernel_spmd(nc, in_maps, core_ids=list(range(N_CORES)))
    return _assemble(res.results)
